# revision 1
# baseline (speedup 1.0000x reference)
"""AttenBlock (InstanceNorm + 1x1-conv QKV self-attention + residual) on 8 trn2 cores.

Problem (hardcoded): x [B=4, C=64, H=96, W=96] f32; wq/wk/wv/wo [64,64]; b* [64].
  h = instance_norm(x); q,k,v = conv1x1(h, w*, b*)
  o = softmax(q^T k / 8) @ v ; out = x + conv1x1(o, wo, bo)

Sharding: 8 cores = 4 samples x 2 query-halves (data parallel, no collectives).
Each core loads its full sample (for norm stats + K/V) plus its query half,
computes attention for its 4608 query rows, returns [64, 4608].

Per-core pipeline (channel-on-partition [C, N] layouts; fp32r matmuls):
  1. bn_stats/bn_aggr -> mean/rstd; h = (x-mean)*rstd as fp32r.
  2. Q,K via PE (M=64); V^T chunks [n,c] via PE with wo FOLDED INTO the V
     weight host-side (wv' = wo@wv), so P@V directly yields the projected
     output. Q/K are duplicated across partition halves with SBUF->SBUF DMA
     so S^T chunk-matmuls (contraction C=64) run two-at-a-time via PE row
     tiling ((0,0)+(64,0)).
  3. Attention as ONE flat software pipeline over (q-block, tile) items
     (tile = 3 S^T chunks = PSUM [128,3,512]): fill matmuls + exp are
     emitted one tile ahead of that tile's P@V matmuls, and a q-block's
     normalization NORM_LAG tiles later still, so the PE FIFO never queues
     behind ScalarE or the reciprocal chain. exp runs on ScalarE straight
     from PSUM at FD=1536 (scale=1/8 folded in; no max-subtraction --
     scores/8 are ~N(0,2), exp stays well inside fp32 range). P@V with
     stationary [V^T | ones] [128,65] accumulates projected-O^T (rows
     0..63) + softmax denominator (row 64) into one PSUM bank per q-block.
  4. normalize: DVE reciprocal of row 64, broadcast across partitions via a
     DRAM-bounce stride-0 DMA, multiply + residual add on DVE.
Bottleneck: ScalarE exp at ~1.2-1.4 cyc/elem from PSUM (~42.5M exps/core);
measured single-shot ~0.39-0.44 ms/core via repeat-loop differential.
"""

import numpy as np

import concourse.bass as bass
import concourse.mybir as mybir
import concourse.tile as tile
from concourse import bacc
from concourse.bass_utils import run_bass_kernel_spmd

F32 = mybir.dt.float32
F32R = mybir.dt.float32r
BF16 = mybir.dt.bfloat16
AF = mybir.ActivationFunctionType
ALU = mybir.AluOpType

C = 64          # channels
CA = C + 1      # channels + ones row (denominator trick)
N = 9216        # H*W
NQ = 4608       # query rows per core
QB = 512        # q-block width
NQB = NQ // QB  # 9
NPAIR = 36      # k-chunk pairs (chunk i pairs with i+36)
EPS = 1e-5

_cache = {}


def _build(use_bias, repeat=1, bench_mode="full", repeat_all=False):
    """repeat>1 wraps the attention phase in a hardware loop (benchmarking
    only -- lets wall-clock deltas between repeat counts expose the true
    per-iteration device time despite ~1.5s of axon dispatch overhead)."""
    nc = bacc.Bacc()
    xs = nc.dram_tensor("xs", [C, N], F32, kind="ExternalInput")
    xq = nc.dram_tensor("xq", [C, NQ], F32, kind="ExternalInput")
    wqt = nc.dram_tensor("wqt", [C, C], F32, kind="ExternalInput")
    wkt = nc.dram_tensor("wkt", [C, C], F32, kind="ExternalInput")
    wvt = nc.dram_tensor("wvt", [C, C], F32, kind="ExternalInput")  # (wo@wv)^T
    bias_in = {}
    if use_bias:
        for nm in ("bq", "bk", "bsum"):
            bias_in[nm] = nc.dram_tensor(nm, [C, 1], F32, kind="ExternalInput")
    out = nc.dram_tensor("out", [C, NQ], F32, kind="ExternalOutput")

    import contextlib as _ctxlib
    with tile.TileContext(nc) as tc:
        with (
            tc.For_i(0, repeat, 1) if repeat > 1 and repeat_all
            else _ctxlib.nullcontext(),
            tc.tile_pool(name="persist", bufs=1) as persist,
            tc.tile_pool(name="attn_sb", bufs=4) as attn_sb,
            tc.tile_pool(name="norm_sb", bufs=2) as norm_sb,
            tc.tile_pool(name="outp_sb", bufs=2) as outp_sb,
            nc.allow_low_precision(reason="fp32r matmul inputs"),
        ):
            # ---------------- phase 0: loads ----------------
            xs_sb = persist.tile([C, N], F32)
            for d in range(4):
                nc.sync.dma_start(xs_sb[:, d * (N // 4):(d + 1) * (N // 4)],
                                  xs[:, d * (N // 4):(d + 1) * (N // 4)])
            xq_sb = persist.tile([C, NQ], F32)
            for d in range(2):
                nc.sync.dma_start(xq_sb[:, d * (NQ // 2):(d + 1) * (NQ // 2)],
                                  xq[:, d * (NQ // 2):(d + 1) * (NQ // 2)])
            wqt_sb = persist.tile([C, C], F32R)
            nc.gpsimd.dma_start(wqt_sb[:], wqt[:])
            wkt_sb = persist.tile([C, C], F32R)
            nc.gpsimd.dma_start(wkt_sb[:], wkt[:])
            wvt_sb = persist.tile([C, C], F32R)
            nc.gpsimd.dma_start(wvt_sb[:], wvt[:])
            bias_sb = {}
            for nm, t in bias_in.items():
                bias_sb[nm] = persist.tile([C, 1], F32, name=nm + "_sb")
                nc.sync.dma_start(bias_sb[nm][:], t[:])
            ones_vt = persist.tile([128, 2 * NPAIR, 1], F32)
            nc.gpsimd.memset(ones_vt[:], 1.0)

            # ---------------- phase 1: instance-norm stats ----------------
            with tc.tile_pool(name="stats", bufs=1) as stats_pool:
                stats = stats_pool.tile([C, N // 512, 6], F32)
                for j in range(N // 512):
                    nc.vector.bn_stats(
                        out=stats[:, j, :], in_=xs_sb[:, j * 512:(j + 1) * 512]
                    )
                mv = stats_pool.tile([C, 2], F32)
                nc.vector.bn_aggr(out=mv[:], in_=stats[:])
                eps_t = stats_pool.tile([C, 1], F32)
                nc.vector.memset(eps_t[:], EPS)
                std = stats_pool.tile([C, 1], F32)
                nc.scalar.activation(std[:], mv[:, 1:2], AF.Sqrt, bias=eps_t[:])
                rstd = stats_pool.tile([C, 1], F32)
                nc.vector.reciprocal(rstd[:], std[:])

                # ---------------- phase 2: normalize ----------------
                h = persist.tile([C, N], F32R)
                nc.vector.tensor_scalar(
                    out=h[:], in0=xs_sb[:],
                    scalar1=mv[:, 0:1], scalar2=rstd[:],
                    op0=ALU.subtract, op1=ALU.mult,
                )
                hq = persist.tile([C, NQ], F32R)
                nc.vector.tensor_scalar(
                    out=hq[:], in0=xq_sb[:],
                    scalar1=mv[:, 0:1], scalar2=rstd[:],
                    op0=ALU.subtract, op1=ALU.mult,
                )

                # ---------------- phase 3: Q, K, V^T ----------------
                # KK: [0:64] = K cols 0:4608, [64:128] = K cols 4608:9216
                # QQ: [0:64] = Q, [64:128] = Q (copy); QQ top doubles as
                # staging for K's high half before Q lands there.
                QQ = persist.tile([128, NQ], F32R)
                KK = persist.tile([128, NQ], F32R)
                VT = persist.tile([128, 2 * NPAIR, CA], F32R)

                copy_tick = [0]

                def psum_to_sbuf(dst, src, bias):
                    if use_bias:
                        nc.vector.tensor_scalar_add(out=dst, in0=src,
                                                    scalar1=bias_sb[bias][:])
                    elif copy_tick[0] % 2 == 0:
                        nc.vector.tensor_copy(dst, src)
                    else:
                        nc.scalar.copy(dst, src)
                    copy_tick[0] += 1

                with tc.tile_pool(name="qkv_ps", bufs=2, space="PSUM") as qkv_ps:
                    for j in range(2 * NQB):  # K over all 9216 cols
                        sl = slice(j * QB, (j + 1) * QB)
                        pk = qkv_ps.tile([C, QB], F32, tag="pk")
                        nc.tensor.matmul(pk[:], wkt_sb[:], h[:, sl],
                                         start=True, stop=True)
                        if j < NQB:
                            psum_to_sbuf(KK[0:C, sl], pk[:], "bk")
                        else:
                            sl2 = slice((j - NQB) * QB, (j - NQB + 1) * QB)
                            psum_to_sbuf(QQ[0:C, sl2], pk[:], "bk")  # staging
                    nc.sync.dma_start(KK[C:128, :], QQ[0:C, :])
                    for j in range(NQB):  # Q over this core's 4608 cols
                        sl = slice(j * QB, (j + 1) * QB)
                        pq = qkv_ps.tile([C, QB], F32, tag="pq")
                        nc.tensor.matmul(pq[:], wqt_sb[:], hq[:, sl],
                                         start=True, stop=True)
                        psum_to_sbuf(QQ[0:C, sl], pq[:], "bq")
                    nc.sync.dma_start(QQ[C:128, :], QQ[0:C, :])
                    for g in range(9):  # V^T chunks [n, c]
                        pv = qkv_ps.tile([128, 8, C], F32, tag="pv")
                        for u in range(8):
                            nb = g * 8 + u
                            nc.tensor.matmul(
                                pv[:, u, :],
                                h[:, nb * 128:(nb + 1) * 128],
                                wvt_sb[:],
                                start=(u == 0), stop=(u == 7),
                            )
                        nc.vector.tensor_copy(VT[:, g * 8:(g + 1) * 8, 0:C], pv[:])
                    nc.vector.tensor_copy(VT[:, :, C:CA], ones_vt[:])

            # ---------------- phase 4: attention ----------------
            import contextlib

            with (
                tc.tile_pool(name="st_ps", bufs=2, space="PSUM") as st_ps,
                tc.tile_pool(name="po_ps", bufs=2, space="PSUM") as po_ps,
                tc.tile_pool(name="dram_nb", bufs=2, space="DRAM") as dram_nb,
                tc.For_i(0, repeat, 1) if repeat > 1 and not repeat_all
                else contextlib.nullcontext(),
            ):
                # chunk c of S^T: rows k in [128c, 128c+128). Chunks 0..35 use
                # the top partition halves of KK/QQ, 36..71 the bottom (row-
                # tiled pair concurrency). One flat software pipeline across
                # all (q-block, tile) items: fill+exp are emitted one tile
                # ahead of that tile's P@V, and a q-block's normalization is
                # emitted NORM_LAG tiles later still -- the PE FIFO never
                # queues behind ScalarE or the reciprocal chain.
                NSLOT = int(__import__("os").environ.get("ATT_NSLOT", "3"))
                NTILE = 2 * NPAIR // NSLOT  # tiles per q-block
                NORM_LAG = 6

                def st_mm(dst, c, qsl):
                    if c < NPAIR:
                        nc.tensor.matmul(dst, KK[0:C, c * 128:(c + 1) * 128],
                                         QQ[0:C, qsl], start=True, stop=True)
                    else:
                        c2 = c - NPAIR
                        nc.tensor.matmul(dst, KK[C:128, c2 * 128:(c2 + 1) * 128],
                                         QQ[C:128, qsl], start=True, stop=True)

                # interleave top/bottom chunks so consecutive fills land on
                # alternating PE row groups (keeps the 2x row-tile overlap)
                chunk_seq = []
                for p in range(NPAIR):
                    chunk_seq += [p, p + NPAIR]

                def tile_chunks(k):
                    return chunk_seq[k * NSLOT:(k + 1) * NSLOT]

                def qsl_of(qb):
                    return slice(qb * QB, (qb + 1) * QB)

                def normalize_and_output(qb, po):
                    # rows 0..63 = O^T unnorm, row 64 = denominator
                    recip = norm_sb.tile([1, QB], F32, tag="recip")
                    if __import__("os").environ.get("ATT_FAST_RECIP"):
                        den = norm_sb.tile([1, QB], F32, tag="den")
                        nc.vector.tensor_copy(den[:], po[C:CA, :])
                        nc.vector.reciprocal_approx_fast(recip[:], den[:])
                    else:
                        nc.vector.reciprocal(recip[:], po[C:CA, :])
                    # broadcast 1/denom across 64 partitions: bounce through
                    # DRAM (SBUF APs cannot have stride-0 partition dim)
                    rscr = dram_nb.tile([1, QB], F32, tag="rscr")
                    nc.sync.dma_start(rscr[:], recip[:])
                    rb = norm_sb.tile([C, QB], F32, tag="rb")
                    rscr_b = bass.AP(tensor=rscr.tensor, offset=rscr[:].offset,
                                     ap=[[0, C]] + list(rscr[:].ap))
                    nc.sync.dma_start(rb[:], rscr_b)
                    # wo is folded into V (host passes wv<-wo@wv), so po rows
                    # 0..63 are already the projected output (unnormalized):
                    # out = x + po*rb (+ wo@bv + bo when biases are nonzero)
                    t1 = norm_sb.tile([C, QB], F32, tag="t1")
                    nc.vector.tensor_mul(t1[:], po[0:C, :], rb[:])
                    ot = outp_sb.tile([C, QB], F32, tag="ot")
                    if use_bias:
                        nc.vector.scalar_tensor_tensor(
                            out=ot[:], in0=t1[:], scalar=bias_sb["bsum"][:],
                            in1=xq_sb[:, qsl_of(qb)], op0=ALU.add, op1=ALU.add,
                        )
                    else:
                        nc.vector.tensor_add(ot[:], t1[:], xq_sb[:, qsl_of(qb)])
                    nc.sync.dma_start(out[:, qsl_of(qb)], ot[:])

                flat = [(qb, k) for qb in range(NQB) for k in range(NTILE)]
                pts = {}
                po_tiles = {}
                for idx in range(len(flat) + 1 + NORM_LAG):
                    # gather this step's PE work: fill MMs for flat[idx] and
                    # P@V MMs for flat[idx-1], interleaved so the PV weight
                    # loads pull ahead while fill MMs stream.
                    fill_mms = []
                    if idx < len(flat):
                        qb, k = flat[idx]
                        st = st_ps.tile([128, NSLOT, QB], F32, tag="st")
                        fill_mms = [(st[:, s, :], c, qsl_of(qb))
                                    for s, c in enumerate(tile_chunks(k))]
                    pv_mms = []
                    pv_idx = idx - 1
                    if bench_mode == "full" and 0 <= pv_idx < len(flat):
                        qb2, k2 = flat[pv_idx]
                        if k2 == 0:
                            po_tiles[qb2] = po_ps.tile([CA, QB], F32, tag="po",
                                                       name="po")
                        po = po_tiles[qb2]
                        pt_prev = pts.pop(pv_idx)
                        for s, c in enumerate(tile_chunks(k2)):
                            first = (k2 == 0 and s == 0)
                            last = (k2 == NTILE - 1 and s == NSLOT - 1)
                            pv_mms.append((po, c, pt_prev, s, first, last))
                    for i in range(max(len(fill_mms), len(pv_mms))):
                        if i < len(fill_mms):
                            dst, c, qsl = fill_mms[i]
                            st_mm(dst, c, qsl)
                        if i < len(pv_mms):
                            po, c, pt_prev, s, first, last = pv_mms[i]
                            nc.tensor.matmul(po[:], VT[:, c, :], pt_prev[:, s, :],
                                             start=first, stop=last)
                    if idx < len(flat) and bench_mode != "st_only":
                        qb, k = flat[idx]
                        pt = attn_sb.tile([128, NSLOT, QB], F32R, tag="pt")
                        nc.scalar.activation(
                            pt[:].rearrange("p a b -> p (a b)"),
                            st[:].rearrange("p a b -> p (a b)"),
                            AF.Exp, scale=0.125)
                        pts[idx] = pt
                    if bench_mode != "full":
                        continue
                    nrm_idx = idx - 1 - NORM_LAG
                    if 0 <= nrm_idx < len(flat):
                        qb, k = flat[nrm_idx]
                        if k == NTILE - 1:
                            normalize_and_output(qb, po_tiles.pop(qb))

    nc.compile()
    return nc


def _get_nc(use_bias):
    key = ("nc", use_bias)
    if key not in _cache:
        _cache[key] = _build(use_bias)
    return _cache[key]


def _make_in_maps(x, wq, bq, wk, bk, wv, bv, wo, bo, use_bias):
    ws = {
        "wqt": np.ascontiguousarray(wq.T.astype(np.float32)),
        "wkt": np.ascontiguousarray(wk.T.astype(np.float32)),
        "wvt": np.ascontiguousarray(
            (wo.astype(np.float64) @ wv.astype(np.float64)).T.astype(np.float32)),
    }
    if use_bias:
        bsum = (wo.astype(np.float64) @ bv.astype(np.float64)
                + bo.astype(np.float64)).astype(np.float32)
        for nm, b in (("bq", bq), ("bk", bk), ("bsum", bsum)):
            ws[nm] = np.ascontiguousarray(b.astype(np.float32).reshape(C, 1))
    in_maps = []
    for core in range(8):
        b, half = core // 2, core % 2
        xsf = np.ascontiguousarray(x[b].reshape(C, N).astype(np.float32))
        xqf = np.ascontiguousarray(xsf[:, half * NQ:(half + 1) * NQ])
        in_maps.append({"xs": xsf, "xq": xqf, **ws})
    return in_maps


def run(inputs, trace=False):
    inputs = {k: np.asarray(v) for k, v in inputs.items()}
    use_bias = any(
        np.any(inputs[nm]) for nm in ("bq", "bk", "bv", "bo")
    )
    nc = _get_nc(use_bias)
    in_maps = _make_in_maps(use_bias=use_bias, **inputs)
    res = run_bass_kernel_spmd(nc, in_maps, list(range(8)), trace=trace)
    B = inputs["x"].shape[0]
    H = W = 96
    full = np.empty((B, C, H, W), dtype=np.float32)
    for core in range(8):
        b, half = core // 2, core % 2
        full[b].reshape(C, N)[:, half * NQ:(half + 1) * NQ] = res.results[core]["out"]
    return full, res


def kernel(**inputs):
    return run(inputs, trace=False)[0]



# revision 15
# speedup vs baseline: 1.6396x; 1.6396x over previous
"""AttenBlock (InstanceNorm + 1x1-conv QKV self-attention + residual) on 8 trn2 cores.

Problem (hardcoded): x [B=4, C=64, H=96, W=96] f32; wq/wk/wv/wo [64,64]; b* [64].
  h = instance_norm(x); q,k,v = conv1x1(h, w*, b*)
  o = softmax(q^T k / 8) @ v ; out = x + conv1x1(o, wo, bo)

Sharding: 8 cores = 4 samples x 2 query-halves (data parallel, no collectives).
Each core loads its full sample (for norm stats + K/V) plus its query half,
computes attention for its 4608 query rows, returns [64, 4608].

Per-core pipeline (channel-on-partition [C, N] layouts; bf16 matmuls):
  1. x DMA in 6 chunks interleaved with bn_stats; rstd = exp(-0.5*ln(var
     +eps)) on ScalarE (Ln+Exp share one activation table set, so the
     attention exps never pay a table switch); h = (x-mean)*rstd as bf16.
  2. Q,K via PE (M=64) with the high partition halves written directly via
     col-tiled matmuls (tile_position=(0,64)) -- no SBUF->SBUF staging
     DMAs. V^T chunks [n,c] via PE with wo FOLDED INTO the V weight
     host-side (wv' = wo@wv), so P@V directly yields the projected output.
     All matmul operands bf16 (weights shipped bf16 from host; FWL halves
     LDWEIGHTS time for the 128-col stationaries).
  3. Attention as ONE flat software pipeline over (q-block, tile) items
     (tile = NSLOT=2 S^T chunks = PSUM [128,2,512] fp32, 3-deep buffer ring
     so fill(t+3) only waits exp(t)). Per idx the PE gets ALL fill matmuls
     for tile t before the P@V matmuls of t-1: PV waits on exp(t-1), and
     keeping fills ahead of that stall in the in-order PE queue lets
     exp(t) start the moment exp(t-1) retires.
     exp is SPLIT across two engines working concurrently:
       - ScalarE: exact exp from PSUM at FD=1024, scale=1/8 folded in
         (no max-subtraction -- scores/8 ~N(0,2.1), fp32 exp never
         overflows);
       - DVE (17 of every 36 tiles): Schraudolph bit-trick exp -- one
         tensor_scalar (s*A+B) written as int16 whose bits ARE
         bf16(exp(s/8)) (~2% rel err, common-mode cancels in softmax;
         end-to-end rel L2 ~3e-3 vs the 2e-2 gate).
     pt tiles are bf16; P@V with stationary [V^T | ones] [128,65] bf16
     accumulates projected-O^T (rows 0..63) + softmax denominator (row 64)
     into one PSUM bank per q-block.
  4. normalize in two lagged stages so the DRAM-bounce broadcast never
     blocks the DVE exp stream: stage A = reciprocal + stride-0 broadcast
     DMAs; stage B (2 tiles later) = multiply + residual add on DVE.
Bottlenecks now balanced: PE (fill+PV streams) ~0.29 ms, ScalarE/DVE exp
~0.21 ms each; measured ~0.33 ms/core single-shot via repeat differential
(baseline before this work: ~0.47-0.51 ms).
"""

import numpy as np

import concourse.bass as bass
import concourse.mybir as mybir
import concourse.tile as tile
from concourse import bacc
from concourse.bass_utils import run_bass_kernel_spmd

F32 = mybir.dt.float32
F32R = mybir.dt.float32r
BF16 = mybir.dt.bfloat16
AF = mybir.ActivationFunctionType
ALU = mybir.AluOpType

C = 64          # channels
CA = C + 1      # channels + ones row (denominator trick)
N = 9216        # H*W
NQ = 4608       # query rows per core
QB = 512        # q-block width
NQB = NQ // QB  # 9
NPAIR = 36      # k-chunk pairs (chunk i pairs with i+36)
EPS = 1e-5

_cache = {}


def _build(use_bias, repeat=1, bench_mode="full", repeat_all=False):
    """repeat>1 wraps the attention phase in a hardware loop (benchmarking
    only -- lets wall-clock deltas between repeat counts expose the true
    per-iteration device time despite ~1.5s of axon dispatch overhead)."""
    nc = bacc.Bacc()
    xs = nc.dram_tensor("xs", [C, N], F32, kind="ExternalInput")
    xq = nc.dram_tensor("xq", [C, NQ], F32, kind="ExternalInput")
    wqt = nc.dram_tensor("wqt", [C, C], BF16, kind="ExternalInput")
    wkt = nc.dram_tensor("wkt", [C, C], BF16, kind="ExternalInput")
    wvt = nc.dram_tensor("wvt", [C, C], BF16, kind="ExternalInput")  # (wo@wv)^T
    bias_in = {}
    if use_bias:
        for nm in ("bq", "bk", "bsum"):
            bias_in[nm] = nc.dram_tensor(nm, [C, 1], F32, kind="ExternalInput")
    out = nc.dram_tensor("out", [C, NQ], F32, kind="ExternalOutput")

    import contextlib as _ctxlib
    with tile.TileContext(nc) as tc:
        with (
            tc.For_i(0, repeat, 1) if repeat > 1 and repeat_all
            else _ctxlib.nullcontext(),
            tc.tile_pool(name="persist", bufs=1) as persist,
            tc.tile_pool(name="attn_sb", bufs=4) as attn_sb,
            tc.tile_pool(name="norm_sb", bufs=2) as norm_sb,
            tc.tile_pool(name="outp_sb", bufs=2) as outp_sb,
            nc.allow_low_precision(reason="fp32r matmul inputs"),
        ):
            # ---------------- phase 0: loads ----------------
            # xs lands in 6 chunks so bn_stats can start on chunk 0 while
            # later chunks are still in flight.
            NXCH = 6
            XCW = N // NXCH  # 1536 cols per chunk (3 bn_stats slices)
            xs_sb = persist.tile([C, N], F32)
            xq_sb = persist.tile([C, NQ], F32)
            for d in range(2):
                nc.sync.dma_start(xq_sb[:, d * (NQ // 2):(d + 1) * (NQ // 2)],
                                  xq[:, d * (NQ // 2):(d + 1) * (NQ // 2)])
            wqt_sb = persist.tile([C, C], BF16)
            nc.gpsimd.dma_start(wqt_sb[:], wqt[:])
            wkt_sb = persist.tile([C, C], BF16)
            nc.gpsimd.dma_start(wkt_sb[:], wkt[:])
            wvt_sb = persist.tile([C, C], BF16)
            nc.gpsimd.dma_start(wvt_sb[:], wvt[:])
            bias_sb = {}
            for nm, t in bias_in.items():
                bias_sb[nm] = persist.tile([C, 1], F32, name=nm + "_sb")
                nc.sync.dma_start(bias_sb[nm][:], t[:])
            ones_vt = persist.tile([128, 2 * NPAIR, 1], BF16)
            nc.gpsimd.memset(ones_vt[:], 1.0)

            # ---------------- phase 1: instance-norm stats ----------------
            with tc.tile_pool(name="stats", bufs=1) as stats_pool:
                stats = stats_pool.tile([C, N // 512, 6], F32)
                for d in range(NXCH):
                    nc.sync.dma_start(xs_sb[:, d * XCW:(d + 1) * XCW],
                                      xs[:, d * XCW:(d + 1) * XCW])
                    for j in range(d * (XCW // 512), (d + 1) * (XCW // 512)):
                        nc.vector.bn_stats(
                            out=stats[:, j, :],
                            in_=xs_sb[:, j * 512:(j + 1) * 512])
                mv = stats_pool.tile([C, 2], F32)
                nc.vector.bn_aggr(out=mv[:], in_=stats[:])
                eps_t = stats_pool.tile([C, 1], F32)
                nc.vector.memset(eps_t[:], EPS)
                # rstd = exp(-0.5*ln(var+eps)): Ln and Exp share one table set
                # (natural_log_exp_and_others) -- avoids the Sqrt-set switch.
                lnv = stats_pool.tile([C, 1], F32)
                nc.scalar.activation(lnv[:], mv[:, 1:2], AF.Ln, bias=eps_t[:])
                rstd = stats_pool.tile([C, 1], F32)
                nc.scalar.activation(rstd[:], lnv[:], AF.Exp, scale=-0.5)

                # ---------------- phase 2: normalize (bf16, in pieces) ----
                h = persist.tile([C, N], BF16)
                for p in range(3):
                    sl = slice(p * (N // 3), (p + 1) * (N // 3))
                    nc.vector.tensor_scalar(
                        out=h[:, sl], in0=xs_sb[:, sl],
                        scalar1=mv[:, 0:1], scalar2=rstd[:],
                        op0=ALU.subtract, op1=ALU.mult,
                    )
                hq = persist.tile([C, NQ], BF16)
                nc.vector.tensor_scalar(
                    out=hq[:], in0=xq_sb[:],
                    scalar1=mv[:, 0:1], scalar2=rstd[:],
                    op0=ALU.subtract, op1=ALU.mult,
                )

                # ---------------- phase 3: Q, K, V^T ----------------
                # KK: [0:64] = K cols 0:4608, [64:128] = K cols 4608:9216
                # QQ: [0:64] = Q, [64:128] = Q. High halves are written
                # directly from PSUM partitions 64:128 (the projection MM is
                # col-tiled to tile_position=(0,64)), so no SBUF->SBUF
                # staging DMAs are needed.
                QQ = persist.tile([128, NQ], BF16)
                KK = persist.tile([128, NQ], BF16)
                VT = persist.tile([128, 2 * NPAIR, CA], BF16)

                copy_tick = [0]

                def psum_to_sbuf(dst, src, bias):
                    if use_bias:
                        nc.vector.tensor_scalar_add(out=dst, in0=src,
                                                    scalar1=bias_sb[bias][:])
                    elif copy_tick[0] % 2 == 0:
                        nc.vector.tensor_copy(dst, src)
                    else:
                        nc.scalar.copy(dst, src)
                    copy_tick[0] += 1

                with tc.tile_pool(name="qkv_ps", bufs=2, space="PSUM") as qkv_ps:
                    for j in range(2 * NQB):  # K over all 9216 cols
                        sl = slice(j * QB, (j + 1) * QB)
                        pk = qkv_ps.tile([128, QB], F32, tag="pk")
                        if j < NQB:
                            nc.tensor.matmul(pk[0:C, :], wkt_sb[:], h[:, sl],
                                             start=True, stop=True)
                            psum_to_sbuf(KK[0:C, sl], pk[0:C, :], "bk")
                        else:
                            sl2 = slice((j - NQB) * QB, (j - NQB + 1) * QB)
                            nc.tensor.matmul(pk[C:128, :], wkt_sb[:], h[:, sl],
                                             start=True, stop=True,
                                             tile_position=(0, 64))
                            psum_to_sbuf(KK[C:128, sl2], pk[C:128, :], "bk")
                    for j in range(NQB):  # Q, both partition halves at once
                        sl = slice(j * QB, (j + 1) * QB)
                        pq = qkv_ps.tile([128, QB], F32, tag="pq")
                        nc.tensor.matmul(pq[0:C, :], wqt_sb[:], hq[:, sl],
                                         start=True, stop=True)
                        nc.tensor.matmul(pq[C:128, :], wqt_sb[:], hq[:, sl],
                                         start=True, stop=True,
                                         tile_position=(0, 64))
                        psum_to_sbuf(QQ[0:C, sl], pq[0:C, :], "bq")
                        psum_to_sbuf(QQ[C:128, sl], pq[C:128, :], "bq")
                    for g in range(9):  # V^T chunks [n, c]
                        pv = qkv_ps.tile([128, 8, C], F32, tag="pv")
                        for u in range(8):
                            nb = g * 8 + u
                            nc.tensor.matmul(
                                pv[:, u, :],
                                h[:, nb * 128:(nb + 1) * 128],
                                wvt_sb[:],
                                start=(u == 0), stop=(u == 7),
                            )
                        if g % 2 == 0:
                            nc.vector.tensor_copy(VT[:, g * 8:(g + 1) * 8, 0:C],
                                                  pv[:])
                        else:
                            nc.scalar.copy(VT[:, g * 8:(g + 1) * 8, 0:C], pv[:])
                    nc.vector.tensor_copy(VT[:, :, C:CA], ones_vt[:])

            # ---------------- phase 4: attention ----------------
            import contextlib

            NSLOT = int(__import__("os").environ.get("ATT_NSLOT", "2"))

            with (
                tc.tile_pool(name="st_ps", bufs=6 // NSLOT,
                             space="PSUM") as st_ps,
                tc.tile_pool(name="po_ps", bufs=2, space="PSUM") as po_ps,
                tc.tile_pool(name="dram_nb", bufs=2, space="DRAM") as dram_nb,
                tc.For_i(0, repeat, 1) if repeat > 1 and not repeat_all
                else contextlib.nullcontext(),
            ):
                # chunk c of S^T: rows k in [128c, 128c+128). Chunks 0..35 use
                # the top partition halves of KK/QQ, 36..71 the bottom (row-
                # tiled pair concurrency). One flat software pipeline across
                # all (q-block, tile) items: fill+exp are emitted one tile
                # ahead of that tile's P@V, and a q-block's normalization is
                # emitted NORM_LAG tiles later still -- the PE FIFO never
                # queues behind ScalarE or the reciprocal chain.
                NTILE = 2 * NPAIR // NSLOT  # tiles per q-block
                NORM_LAG = 6
                # exp-engine split: DVE handles N_DVE of every NTILE tiles via
                # the Schraudolph bit-trick (int32 write of s*A+B, bits read
                # back as fp32 ~= exp(s/8); ~2% rel err, cancels in softmax),
                # the rest run exact exp on ScalarE. Both engines stream
                # concurrently, so the exp phase is no longer ScalarE-bound.
                N_DVE = int(__import__("os").environ.get(
                    "ATT_NDVE", "17" if NSLOT == 2 else "10"))
                LOG2E = 1.4426950408889634
                SCH_A = float(np.float32(0.125 * LOG2E * (1 << 7)))
                SCH_B = float(np.float32(127.0 * (1 << 7) - 7.42))

                def exp_on_dve(k):
                    # Bresenham spread of N_DVE dve-tiles over NTILE, avoiding
                    # the last tile (DVE must be free for normalize there)
                    if k == NTILE - 1:
                        return False
                    return (k * N_DVE) // (NTILE - 1) != ((k + 1) * N_DVE) // (NTILE - 1)

                def st_mm(dst, c, qsl):
                    if c < NPAIR:
                        nc.tensor.matmul(dst, KK[0:C, c * 128:(c + 1) * 128],
                                         QQ[0:C, qsl], start=True, stop=True)
                    else:
                        c2 = c - NPAIR
                        nc.tensor.matmul(dst, KK[C:128, c2 * 128:(c2 + 1) * 128],
                                         QQ[C:128, qsl], start=True, stop=True)

                # interleave top/bottom chunks so consecutive fills land on
                # alternating PE row groups (keeps the 2x row-tile overlap)
                chunk_seq = []
                for p in range(NPAIR):
                    chunk_seq += [p, p + NPAIR]

                def tile_chunks(k):
                    return chunk_seq[k * NSLOT:(k + 1) * NSLOT]

                def qsl_of(qb):
                    return slice(qb * QB, (qb + 1) * QB)

                # normalize is split in two lagged stages so the DRAM-bounce
                # broadcast latency never blocks the DVE queue (which also
                # streams exp tiles now): stage A computes 1/denom and kicks
                # off the bounce DMAs; stage B (2 tiles later) multiplies and
                # adds the residual once the broadcast has landed.
                rb_tiles = {}

                def normalize_stage_a(qb, po):
                    # rows 0..63 = O^T unnorm, row 64 = denominator
                    recip = norm_sb.tile([1, QB], F32, tag="recip")
                    nc.vector.reciprocal(recip[:], po[C:CA, :])
                    # broadcast 1/denom across 64 partitions: bounce through
                    # DRAM (SBUF APs cannot have stride-0 partition dim)
                    rscr = dram_nb.tile([1, QB], F32, tag="rscr")
                    nc.sync.dma_start(rscr[:], recip[:])
                    rb = norm_sb.tile([C, QB], F32, tag="rb")
                    rscr_b = bass.AP(tensor=rscr.tensor, offset=rscr[:].offset,
                                     ap=[[0, C]] + list(rscr[:].ap))
                    nc.sync.dma_start(rb[:], rscr_b)
                    rb_tiles[qb] = rb

                def normalize_stage_b(qb, po):
                    # wo is folded into V (host passes wv<-wo@wv), so po rows
                    # 0..63 are already the projected output (unnormalized):
                    # out = x + po*rb (+ wo@bv + bo when biases are nonzero)
                    rb = rb_tiles.pop(qb)
                    t1 = norm_sb.tile([C, QB], F32, tag="t1")
                    nc.vector.tensor_mul(t1[:], po[0:C, :], rb[:])
                    ot = outp_sb.tile([C, QB], F32, tag="ot")
                    if use_bias:
                        nc.vector.scalar_tensor_tensor(
                            out=ot[:], in0=t1[:], scalar=bias_sb["bsum"][:],
                            in1=xq_sb[:, qsl_of(qb)], op0=ALU.add, op1=ALU.add,
                        )
                    else:
                        nc.vector.tensor_add(ot[:], t1[:], xq_sb[:, qsl_of(qb)])
                    nc.sync.dma_start(out[:, qsl_of(qb)], ot[:])

                flat = [(qb, k) for qb in range(NQB) for k in range(NTILE)]
                pts = {}
                po_tiles = {}
                for idx in range(len(flat) + 3 + NORM_LAG):
                    # PE order: ALL fill MMs for flat[idx] first, THEN the P@V
                    # MMs for flat[idx-1]. PV waits on exp(idx-1); emitting the
                    # fills first keeps them out from behind that stall in the
                    # in-order PE queue, so exp(idx) can start the moment
                    # exp(idx-1) retires (fills run during the previous exp).
                    if idx < len(flat):
                        qb, k = flat[idx]
                        st = st_ps.tile([128, NSLOT, QB], F32, tag="st")
                        for s, c in enumerate(tile_chunks(k)):
                            st_mm(st[:, s, :], c, qsl_of(qb))
                    pv_idx = idx - 1
                    if bench_mode == "full" and 0 <= pv_idx < len(flat):
                        qb2, k2 = flat[pv_idx]
                        if k2 == 0:
                            po_tiles[qb2] = po_ps.tile([CA, QB], F32, tag="po",
                                                       name="po")
                        po = po_tiles[qb2]
                        pt_prev = pts.pop(pv_idx)
                        for s, c in enumerate(tile_chunks(k2)):
                            first = (k2 == 0 and s == 0)
                            last = (k2 == NTILE - 1 and s == NSLOT - 1)
                            nc.tensor.matmul(po[:], VT[:, c, :], pt_prev[:, s, :],
                                             start=first, stop=last)
                    if idx < len(flat) and bench_mode != "st_only":
                        qb, k = flat[idx]
                        pt = attn_sb.tile([128, NSLOT, QB], BF16, tag="pt")
                        if exp_on_dve(k):
                            nc.vector.tensor_scalar(
                                out=pt[:].rearrange("p a b -> p (a b)").bitcast(
                                    mybir.dt.int16),
                                in0=st[:].rearrange("p a b -> p (a b)"),
                                scalar1=SCH_A, scalar2=SCH_B,
                                op0=ALU.mult, op1=ALU.add)
                        else:
                            nc.scalar.activation(
                                pt[:].rearrange("p a b -> p (a b)"),
                                st[:].rearrange("p a b -> p (a b)"),
                                AF.Exp, scale=0.125)
                        pts[idx] = pt
                    if bench_mode != "full":
                        continue
                    nrm_idx = idx - 1 - NORM_LAG
                    if 0 <= nrm_idx < len(flat):
                        qb, k = flat[nrm_idx]
                        if k == NTILE - 1:
                            normalize_stage_a(qb, po_tiles[qb])
                    nrm_idx_b = idx - 3 - NORM_LAG
                    if 0 <= nrm_idx_b < len(flat):
                        qb, k = flat[nrm_idx_b]
                        if k == NTILE - 1:
                            normalize_stage_b(qb, po_tiles.pop(qb))

    nc.compile()
    return nc


def _get_nc(use_bias):
    key = ("nc", use_bias)
    if key not in _cache:
        _cache[key] = _build(use_bias)
    return _cache[key]


def _make_in_maps(x, wq, bq, wk, bk, wv, bv, wo, bo, use_bias):
    bf16 = mybir.dt.np(BF16)
    ws = {
        "wqt": np.ascontiguousarray(wq.T.astype(np.float32)).astype(bf16),
        "wkt": np.ascontiguousarray(wk.T.astype(np.float32)).astype(bf16),
        "wvt": np.ascontiguousarray(
            (wo.astype(np.float64) @ wv.astype(np.float64)).T.astype(np.float32)
        ).astype(bf16),
    }
    if use_bias:
        bsum = (wo.astype(np.float64) @ bv.astype(np.float64)
                + bo.astype(np.float64)).astype(np.float32)
        for nm, b in (("bq", bq), ("bk", bk), ("bsum", bsum)):
            ws[nm] = np.ascontiguousarray(b.astype(np.float32).reshape(C, 1))
    in_maps = []
    for core in range(8):
        b, half = core // 2, core % 2
        xsf = np.ascontiguousarray(x[b].reshape(C, N).astype(np.float32))
        xqf = np.ascontiguousarray(xsf[:, half * NQ:(half + 1) * NQ])
        in_maps.append({"xs": xsf, "xq": xqf, **ws})
    return in_maps


def run(inputs, trace=False):
    inputs = {k: np.asarray(v) for k, v in inputs.items()}
    use_bias = any(
        np.any(inputs[nm]) for nm in ("bq", "bk", "bv", "bo")
    )
    nc = _get_nc(use_bias)
    in_maps = _make_in_maps(use_bias=use_bias, **inputs)
    res = run_bass_kernel_spmd(nc, in_maps, list(range(8)), trace=trace)
    B = inputs["x"].shape[0]
    H = W = 96
    full = np.empty((B, C, H, W), dtype=np.float32)
    for core in range(8):
        b, half = core // 2, core % 2
        full[b].reshape(C, N)[:, half * NQ:(half + 1) * NQ] = res.results[core]["out"]
    return full, res


def kernel(**inputs):
    return run(inputs, trace=False)[0]



# revision 17
# speedup vs baseline: 1.6654x; 1.0157x over previous
"""AttenBlock (InstanceNorm + 1x1-conv QKV self-attention + residual) on 8 trn2 cores.

Problem (hardcoded): x [B=4, C=64, H=96, W=96] f32; wq/wk/wv/wo [64,64]; b* [64].
  h = instance_norm(x); q,k,v = conv1x1(h, w*, b*)
  o = softmax(q^T k / 8) @ v ; out = x + conv1x1(o, wo, bo)

Sharding: 8 cores = 4 samples x 2 query-halves (data parallel, no collectives).
Each core loads its full sample (for norm stats + K/V) plus its query half,
computes attention for its 4608 query rows, returns [64, 4608].

Per-core pipeline (channel-on-partition [C, N] layouts; bf16 matmuls):
  1. x DMA in 6 chunks interleaved with bn_stats; rstd = exp(-0.5*ln(var
     +eps)) on ScalarE (Ln+Exp share one activation table set, so the
     attention exps never pay a table switch); h = (x-mean)*rstd as bf16.
  2. Q,K via PE (M=64) with the high partition halves written directly via
     col-tiled matmuls (tile_position=(0,64)) -- no SBUF->SBUF staging
     DMAs. V^T chunks [n,c] via PE with wo FOLDED INTO the V weight
     host-side (wv' = wo@wv), so P@V directly yields the projected output.
     All matmul operands bf16 (weights shipped bf16 from host; FWL halves
     LDWEIGHTS time for the 128-col stationaries).
  3. Attention as ONE flat software pipeline over (q-block, tile) items
     (tile = NSLOT=2 S^T chunks = PSUM [128,2,512] fp32, 3-deep buffer ring
     so fill(t+3) only waits exp(t)). Per idx the PE gets ALL fill matmuls
     for tile t before the P@V matmuls of t-1: PV waits on exp(t-1), and
     keeping fills ahead of that stall in the in-order PE queue lets
     exp(t) start the moment exp(t-1) retires.
     exp is SPLIT across two engines working concurrently:
       - ScalarE: exact exp from PSUM at FD=1024, scale=1/8 folded in
         (no max-subtraction -- scores/8 ~N(0,2.1), fp32 exp never
         overflows);
       - DVE (14 of every 36 tiles): Schraudolph bit-trick exp -- one
         tensor_scalar (s*A+B) written as int16 whose bits ARE
         bf16(exp(s/8)) (~2% rel err, common-mode cancels in softmax;
         end-to-end rel L2 ~3e-3 vs the 2e-2 gate).
     pt tiles are bf16; P@V with stationary [V^T | ones] [128,65] bf16
     accumulates projected-O^T (rows 0..63) + softmax denominator (row 64)
     into one PSUM bank per q-block.
  4. normalize in two lagged stages so the DRAM-bounce broadcast never
     blocks the DVE exp stream: stage A = reciprocal + stride-0 broadcast
     DMAs; stage B (2 tiles later) = multiply + residual add on DVE.
Bottlenecks now balanced: PE (fill+PV streams) ~0.29 ms, ScalarE/DVE exp
~0.21 ms each; measured ~0.33 ms/core single-shot via repeat differential
(baseline before this work: ~0.47-0.51 ms).
"""

import numpy as np

import concourse.bass as bass
import concourse.mybir as mybir
import concourse.tile as tile
from concourse import bacc
from concourse.bass_utils import run_bass_kernel_spmd

F32 = mybir.dt.float32
F32R = mybir.dt.float32r
BF16 = mybir.dt.bfloat16
AF = mybir.ActivationFunctionType
ALU = mybir.AluOpType

C = 64          # channels
CA = C + 1      # channels + ones row (denominator trick)
N = 9216        # H*W
NQ = 4608       # query rows per core
QB = 512        # q-block width
NQB = NQ // QB  # 9
NPAIR = 36      # k-chunk pairs (chunk i pairs with i+36)
EPS = 1e-5

_cache = {}


def _build(use_bias, repeat=1, bench_mode="full", repeat_all=False):
    """repeat>1 wraps the attention phase in a hardware loop (benchmarking
    only -- lets wall-clock deltas between repeat counts expose the true
    per-iteration device time despite ~1.5s of axon dispatch overhead)."""
    nc = bacc.Bacc()
    xs = nc.dram_tensor("xs", [C, N], F32, kind="ExternalInput")
    xq = nc.dram_tensor("xq", [C, NQ], F32, kind="ExternalInput")
    wqt = nc.dram_tensor("wqt", [C, C], BF16, kind="ExternalInput")
    wkt = nc.dram_tensor("wkt", [C, C], BF16, kind="ExternalInput")
    wvt = nc.dram_tensor("wvt", [C, C], BF16, kind="ExternalInput")  # (wo@wv)^T
    bias_in = {}
    if use_bias:
        for nm in ("bq", "bk", "bsum"):
            bias_in[nm] = nc.dram_tensor(nm, [C, 1], F32, kind="ExternalInput")
    out = nc.dram_tensor("out", [C, NQ], F32, kind="ExternalOutput")

    import contextlib as _ctxlib
    with tile.TileContext(nc) as tc:
        with (
            tc.For_i(0, repeat, 1) if repeat > 1 and repeat_all
            else _ctxlib.nullcontext(),
            tc.tile_pool(name="persist", bufs=1) as persist,
            tc.tile_pool(name="attn_sb", bufs=int(
                __import__("os").environ.get("ATT_PTBUFS", "4"))) as attn_sb,
            tc.tile_pool(name="norm_sb", bufs=2) as norm_sb,
            tc.tile_pool(name="outp_sb", bufs=2) as outp_sb,
            nc.allow_low_precision(reason="fp32r matmul inputs"),
        ):
            # ---------------- phase 0: loads ----------------
            # xs lands in 6 chunks so bn_stats can start on chunk 0 while
            # later chunks are still in flight.
            NXCH = 6
            XCW = N // NXCH  # 1536 cols per chunk (3 bn_stats slices)
            xs_sb = persist.tile([C, N], F32)
            xq_sb = persist.tile([C, NQ], F32)
            for d in range(2):
                nc.sync.dma_start(xq_sb[:, d * (NQ // 2):(d + 1) * (NQ // 2)],
                                  xq[:, d * (NQ // 2):(d + 1) * (NQ // 2)])
            wqt_sb = persist.tile([C, C], BF16)
            nc.gpsimd.dma_start(wqt_sb[:], wqt[:])
            wkt_sb = persist.tile([C, C], BF16)
            nc.gpsimd.dma_start(wkt_sb[:], wkt[:])
            wvt_sb = persist.tile([C, C], BF16)
            nc.gpsimd.dma_start(wvt_sb[:], wvt[:])
            bias_sb = {}
            for nm, t in bias_in.items():
                bias_sb[nm] = persist.tile([C, 1], F32, name=nm + "_sb")
                nc.sync.dma_start(bias_sb[nm][:], t[:])
            ones_vt = persist.tile([128, 2 * NPAIR, 1], BF16)
            nc.gpsimd.memset(ones_vt[:], 1.0)

            # ---------------- phase 1: instance-norm stats ----------------
            with tc.tile_pool(name="stats", bufs=1) as stats_pool:
                stats = stats_pool.tile([C, N // 512, 6], F32)
                for d in range(NXCH):
                    nc.sync.dma_start(xs_sb[:, d * XCW:(d + 1) * XCW],
                                      xs[:, d * XCW:(d + 1) * XCW])
                    for j in range(d * (XCW // 512), (d + 1) * (XCW // 512)):
                        nc.vector.bn_stats(
                            out=stats[:, j, :],
                            in_=xs_sb[:, j * 512:(j + 1) * 512])
                mv = stats_pool.tile([C, 2], F32)
                nc.vector.bn_aggr(out=mv[:], in_=stats[:])
                eps_t = stats_pool.tile([C, 1], F32)
                nc.vector.memset(eps_t[:], EPS)
                # rstd = exp(-0.5*ln(var+eps)): Ln and Exp share one table set
                # (natural_log_exp_and_others) -- avoids the Sqrt-set switch.
                lnv = stats_pool.tile([C, 1], F32)
                nc.scalar.activation(lnv[:], mv[:, 1:2], AF.Ln, bias=eps_t[:])
                rstd = stats_pool.tile([C, 1], F32)
                nc.scalar.activation(rstd[:], lnv[:], AF.Exp, scale=-0.5)

                # ---------------- phase 2: normalize (bf16, in pieces) ----
                h = persist.tile([C, N], BF16)
                for p in range(3):
                    sl = slice(p * (N // 3), (p + 1) * (N // 3))
                    nc.vector.tensor_scalar(
                        out=h[:, sl], in0=xs_sb[:, sl],
                        scalar1=mv[:, 0:1], scalar2=rstd[:],
                        op0=ALU.subtract, op1=ALU.mult,
                    )
                hq = persist.tile([C, NQ], BF16)
                nc.vector.tensor_scalar(
                    out=hq[:], in0=xq_sb[:],
                    scalar1=mv[:, 0:1], scalar2=rstd[:],
                    op0=ALU.subtract, op1=ALU.mult,
                )

                # ---------------- phase 3: Q, K, V^T ----------------
                # KK: [0:64] = K cols 0:4608, [64:128] = K cols 4608:9216
                # QQ: [0:64] = Q, [64:128] = Q. High halves are written
                # directly from PSUM partitions 64:128 (the projection MM is
                # col-tiled to tile_position=(0,64)), so no SBUF->SBUF
                # staging DMAs are needed.
                QQ = persist.tile([128, NQ], BF16)
                KK = persist.tile([128, NQ], BF16)
                VT = persist.tile([128, 2 * NPAIR, CA], BF16)

                copy_tick = [0]

                def psum_to_sbuf(dst, src, bias):
                    if use_bias:
                        nc.vector.tensor_scalar_add(out=dst, in0=src,
                                                    scalar1=bias_sb[bias][:])
                    elif copy_tick[0] % 2 == 0:
                        nc.vector.tensor_copy(dst, src)
                    else:
                        nc.scalar.copy(dst, src)
                    copy_tick[0] += 1

                with tc.tile_pool(name="qkv_ps", bufs=2, space="PSUM") as qkv_ps:
                    for j in range(2 * NQB):  # K over all 9216 cols
                        sl = slice(j * QB, (j + 1) * QB)
                        pk = qkv_ps.tile([128, QB], F32, tag="pk")
                        if j < NQB:
                            nc.tensor.matmul(pk[0:C, :], wkt_sb[:], h[:, sl],
                                             start=True, stop=True)
                            psum_to_sbuf(KK[0:C, sl], pk[0:C, :], "bk")
                        else:
                            sl2 = slice((j - NQB) * QB, (j - NQB + 1) * QB)
                            nc.tensor.matmul(pk[C:128, :], wkt_sb[:], h[:, sl],
                                             start=True, stop=True,
                                             tile_position=(0, 64))
                            psum_to_sbuf(KK[C:128, sl2], pk[C:128, :], "bk")
                    for j in range(NQB):  # Q, both partition halves at once
                        sl = slice(j * QB, (j + 1) * QB)
                        pq = qkv_ps.tile([128, QB], F32, tag="pq")
                        nc.tensor.matmul(pq[0:C, :], wqt_sb[:], hq[:, sl],
                                         start=True, stop=True)
                        nc.tensor.matmul(pq[C:128, :], wqt_sb[:], hq[:, sl],
                                         start=True, stop=True,
                                         tile_position=(0, 64))
                        psum_to_sbuf(QQ[0:C, sl], pq[0:C, :], "bq")
                        psum_to_sbuf(QQ[C:128, sl], pq[C:128, :], "bq")
                    for g in range(9):  # V^T chunks [n, c]
                        pv = qkv_ps.tile([128, 8, C], F32, tag="pv")
                        for u in range(8):
                            nb = g * 8 + u
                            nc.tensor.matmul(
                                pv[:, u, :],
                                h[:, nb * 128:(nb + 1) * 128],
                                wvt_sb[:],
                                start=(u == 0), stop=(u == 7),
                            )
                        if g % 2 == 0:
                            nc.vector.tensor_copy(VT[:, g * 8:(g + 1) * 8, 0:C],
                                                  pv[:])
                        else:
                            nc.scalar.copy(VT[:, g * 8:(g + 1) * 8, 0:C], pv[:])
                    nc.vector.tensor_copy(VT[:, :, C:CA], ones_vt[:])

            # ---------------- phase 4: attention ----------------
            import contextlib

            NSLOT = int(__import__("os").environ.get("ATT_NSLOT", "2"))

            with (
                tc.tile_pool(name="st_ps", bufs=6 // NSLOT,
                             space="PSUM") as st_ps,
                tc.tile_pool(name="po_ps", bufs=2, space="PSUM") as po_ps,
                tc.tile_pool(name="dram_nb", bufs=2, space="DRAM") as dram_nb,
                tc.For_i(0, repeat, 1) if repeat > 1 and not repeat_all
                else contextlib.nullcontext(),
            ):
                # chunk c of S^T: rows k in [128c, 128c+128). Chunks 0..35 use
                # the top partition halves of KK/QQ, 36..71 the bottom (row-
                # tiled pair concurrency). One flat software pipeline across
                # all (q-block, tile) items: fill+exp are emitted one tile
                # ahead of that tile's P@V, and a q-block's normalization is
                # emitted NORM_LAG tiles later still -- the PE FIFO never
                # queues behind ScalarE or the reciprocal chain.
                NTILE = 2 * NPAIR // NSLOT  # tiles per q-block
                NORM_LAG = 6
                # exp-engine split: DVE handles N_DVE of every NTILE tiles via
                # the Schraudolph bit-trick (int32 write of s*A+B, bits read
                # back as fp32 ~= exp(s/8); ~2% rel err, cancels in softmax),
                # the rest run exact exp on ScalarE. Both engines stream
                # concurrently, so the exp phase is no longer ScalarE-bound.
                N_DVE = int(__import__("os").environ.get(
                    "ATT_NDVE", "14" if NSLOT == 2 else "10"))
                LOG2E = 1.4426950408889634
                SCH_A = float(np.float32(0.125 * LOG2E * (1 << 7)))
                SCH_B = float(np.float32(127.0 * (1 << 7) - 7.42))

                def exp_on_dve(k):
                    # Bresenham spread of N_DVE dve-tiles over NTILE, avoiding
                    # the last tile (DVE must be free for normalize there)
                    if k == NTILE - 1:
                        return False
                    return (k * N_DVE) // (NTILE - 1) != ((k + 1) * N_DVE) // (NTILE - 1)

                def st_mm(dst, c, qsl):
                    if c < NPAIR:
                        nc.tensor.matmul(dst, KK[0:C, c * 128:(c + 1) * 128],
                                         QQ[0:C, qsl], start=True, stop=True)
                    else:
                        c2 = c - NPAIR
                        nc.tensor.matmul(dst, KK[C:128, c2 * 128:(c2 + 1) * 128],
                                         QQ[C:128, qsl], start=True, stop=True)

                # interleave top/bottom chunks so consecutive fills land on
                # alternating PE row groups (keeps the 2x row-tile overlap)
                chunk_seq = []
                for p in range(NPAIR):
                    chunk_seq += [p, p + NPAIR]

                def tile_chunks(k):
                    return chunk_seq[k * NSLOT:(k + 1) * NSLOT]

                def qsl_of(qb):
                    return slice(qb * QB, (qb + 1) * QB)

                # normalize is split in two lagged stages so the DRAM-bounce
                # broadcast latency never blocks the DVE queue (which also
                # streams exp tiles now): stage A computes 1/denom and kicks
                # off the bounce DMAs; stage B (2 tiles later) multiplies and
                # adds the residual once the broadcast has landed.
                rb_tiles = {}

                def normalize_stage_a(qb, po):
                    # rows 0..63 = O^T unnorm, row 64 = denominator
                    recip = norm_sb.tile([1, QB], F32, tag="recip")
                    nc.vector.reciprocal(recip[:], po[C:CA, :])
                    # broadcast 1/denom across 64 partitions: bounce through
                    # DRAM (SBUF APs cannot have stride-0 partition dim)
                    rscr = dram_nb.tile([1, QB], F32, tag="rscr")
                    nc.sync.dma_start(rscr[:], recip[:])
                    rb = norm_sb.tile([C, QB], F32, tag="rb")
                    rscr_b = bass.AP(tensor=rscr.tensor, offset=rscr[:].offset,
                                     ap=[[0, C]] + list(rscr[:].ap))
                    nc.sync.dma_start(rb[:], rscr_b)
                    rb_tiles[qb] = rb

                def normalize_stage_b(qb, po):
                    # wo is folded into V (host passes wv<-wo@wv), so po rows
                    # 0..63 are already the projected output (unnormalized):
                    # out = x + po*rb (+ wo@bv + bo when biases are nonzero)
                    rb = rb_tiles.pop(qb)
                    t1 = norm_sb.tile([C, QB], F32, tag="t1")
                    nc.vector.tensor_mul(t1[:], po[0:C, :], rb[:])
                    ot = outp_sb.tile([C, QB], F32, tag="ot")
                    if use_bias:
                        nc.vector.scalar_tensor_tensor(
                            out=ot[:], in0=t1[:], scalar=bias_sb["bsum"][:],
                            in1=xq_sb[:, qsl_of(qb)], op0=ALU.add, op1=ALU.add,
                        )
                    else:
                        nc.vector.tensor_add(ot[:], t1[:], xq_sb[:, qsl_of(qb)])
                    nc.sync.dma_start(out[:, qsl_of(qb)], ot[:])

                flat = [(qb, k) for qb in range(NQB) for k in range(NTILE)]
                pts = {}
                po_tiles = {}
                for idx in range(len(flat) + 3 + NORM_LAG):
                    # PE order: ALL fill MMs for flat[idx] first, THEN the P@V
                    # MMs for flat[idx-1]. PV waits on exp(idx-1); emitting the
                    # fills first keeps them out from behind that stall in the
                    # in-order PE queue, so exp(idx) can start the moment
                    # exp(idx-1) retires (fills run during the previous exp).
                    if idx < len(flat):
                        qb, k = flat[idx]
                        st = st_ps.tile([128, NSLOT, QB], F32, tag="st")
                        for s, c in enumerate(tile_chunks(k)):
                            st_mm(st[:, s, :], c, qsl_of(qb))
                    pv_idx = idx - 1
                    if bench_mode == "full" and 0 <= pv_idx < len(flat):
                        qb2, k2 = flat[pv_idx]
                        if k2 == 0:
                            po_tiles[qb2] = po_ps.tile([CA, QB], F32, tag="po",
                                                       name="po")
                        po = po_tiles[qb2]
                        pt_prev = pts.pop(pv_idx)
                        for s, c in enumerate(tile_chunks(k2)):
                            first = (k2 == 0 and s == 0)
                            last = (k2 == NTILE - 1 and s == NSLOT - 1)
                            nc.tensor.matmul(po[:], VT[:, c, :], pt_prev[:, s, :],
                                             start=first, stop=last)
                    if idx < len(flat) and bench_mode != "st_only":
                        qb, k = flat[idx]
                        pt = attn_sb.tile([128, NSLOT, QB], BF16, tag="pt")
                        if exp_on_dve(k):
                            nc.vector.tensor_scalar(
                                out=pt[:].rearrange("p a b -> p (a b)").bitcast(
                                    mybir.dt.int16),
                                in0=st[:].rearrange("p a b -> p (a b)"),
                                scalar1=SCH_A, scalar2=SCH_B,
                                op0=ALU.mult, op1=ALU.add)
                        else:
                            nc.scalar.activation(
                                pt[:].rearrange("p a b -> p (a b)"),
                                st[:].rearrange("p a b -> p (a b)"),
                                AF.Exp, scale=0.125)
                        pts[idx] = pt
                    if bench_mode != "full":
                        continue
                    nrm_idx = idx - 1 - NORM_LAG
                    if 0 <= nrm_idx < len(flat):
                        qb, k = flat[nrm_idx]
                        if k == NTILE - 1:
                            normalize_stage_a(qb, po_tiles[qb])
                    nrm_idx_b = idx - 3 - NORM_LAG
                    if 0 <= nrm_idx_b < len(flat):
                        qb, k = flat[nrm_idx_b]
                        if k == NTILE - 1:
                            normalize_stage_b(qb, po_tiles.pop(qb))

    nc.compile()
    return nc


def _get_nc(use_bias):
    key = ("nc", use_bias)
    if key not in _cache:
        _cache[key] = _build(use_bias)
    return _cache[key]


def _make_in_maps(x, wq, bq, wk, bk, wv, bv, wo, bo, use_bias):
    bf16 = mybir.dt.np(BF16)
    ws = {
        "wqt": np.ascontiguousarray(wq.T.astype(np.float32)).astype(bf16),
        "wkt": np.ascontiguousarray(wk.T.astype(np.float32)).astype(bf16),
        "wvt": np.ascontiguousarray(
            (wo.astype(np.float64) @ wv.astype(np.float64)).T.astype(np.float32)
        ).astype(bf16),
    }
    if use_bias:
        bsum = (wo.astype(np.float64) @ bv.astype(np.float64)
                + bo.astype(np.float64)).astype(np.float32)
        for nm, b in (("bq", bq), ("bk", bk), ("bsum", bsum)):
            ws[nm] = np.ascontiguousarray(b.astype(np.float32).reshape(C, 1))
    in_maps = []
    for core in range(8):
        b, half = core // 2, core % 2
        xsf = np.ascontiguousarray(x[b].reshape(C, N).astype(np.float32))
        xqf = np.ascontiguousarray(xsf[:, half * NQ:(half + 1) * NQ])
        in_maps.append({"xs": xsf, "xq": xqf, **ws})
    return in_maps


def run(inputs, trace=False):
    inputs = {k: np.asarray(v) for k, v in inputs.items()}
    use_bias = any(
        np.any(inputs[nm]) for nm in ("bq", "bk", "bv", "bo")
    )
    nc = _get_nc(use_bias)
    in_maps = _make_in_maps(use_bias=use_bias, **inputs)
    res = run_bass_kernel_spmd(nc, in_maps, list(range(8)), trace=trace)
    B = inputs["x"].shape[0]
    H = W = 96
    full = np.empty((B, C, H, W), dtype=np.float32)
    for core in range(8):
        b, half = core // 2, core % 2
        full[b].reshape(C, N)[:, half * NQ:(half + 1) * NQ] = res.results[core]["out"]
    return full, res


def kernel(**inputs):
    return run(inputs, trace=False)[0]

